# revision 13
# baseline (speedup 1.0000x reference)
"""Trainium2 Bass kernel for nn_Attention_45724221833663 (sparse_attention).

Strategy: data-parallel over batch B=8 across the 8 NeuronCores (one batch
element per core).  Matmul dtype mix: the q/k projections run in fp8e4
DoubleRow (2 contraction rows per PE cell -> half the matmul passes), the
softmax weights E are stored fp8 (AV = fp8 stationary x bf16 moving), and
everything whose error would reach the output directly (v projection,
scores operands, AV values, output projection) stays bf16 with fp32 PSUM.
Validated end-to-end: amax rel err ~8e-3 vs fp32 reference (tol 2e-2).

Per-core dataflow (host pre-transposes weights/x while sharding; ln_g is
folded into Wp, ln_b into bp):
  xcat8  [c=1024, kvp=1152] fp8  (concat(x_text,x).T, q/k weights x256 on
         host, the 1/65536 is folded into the exp scale)
  xcatT  [c, kvp] bf16           (v projection only)
  vw     [kvp, h, 65] bf16 = (xcatT.T @ WvT) per head + ones column
  qT/kT via DoubleRow fp8: 4 passes of K=256 instead of 8 of K=128
  per head pair (even head on PE row-tile 0, odd on row-tile 64):
    scores for both heads go into ONE [128,1024] psum tile per (kv-tile,
    n-half): disjoint PE row groups + disjoint psum banks -> the two
    matmuls execute concurrently.
    E = exp(scoresT/(8*65536)) -> fp8 e2 tile; kv=0 row and pad rows zeroed
    avp[n,0:65] = sum_kv E[kv,..] @ vw[kv,h,:]  (col 64 = S[n])
    attn[n, h*64:+64] = avp[:, :64]*(1/S) + tanh(g_h)*v_h[kv=0]
  Final head pair runs n-tile-major with LN stats + output projection
  interleaved.  The projection consumes RAW attn (transposed via PE), and
  LayerNorm is applied algebraically at psum evacuation:
      out = (attnT @ Wpf) * rstd - (mu*rstd*colsum(Wpf) - bp)
  so the PE transpose/matmul chain never waits on the LN stats.  rstd is
  computed as exp(-0.5*ln(var+eps)) so ScalarE stays on the
  natural_log_exp table set (no mid-kernel table switch).  Transposes are
  hoisted 2 chunks ahead of their matmuls and their psum->sbuf copies run
  on ScalarE (idle there: all exps are done by that phase).  Output is
  written bf16 and upcast on the host.
"""

import os
import numpy as np
import ml_dtypes

import concourse.bacc as bacc
import concourse.tile as tile
from concourse import mybir
from concourse.masks import make_identity
from concourse.bass_utils import run_bass_kernel_spmd

F32 = mybir.dt.float32
BF16 = mybir.dt.bfloat16
E4 = mybir.dt.float8e4
AF = mybir.ActivationFunctionType
OP = mybir.AluOpType
DRMODE = mybir.MatmulPerfMode.DoubleRow

B, N, P, DIM, H = 8, 1024, 77, 1024, 16
HD = DIM // H          # 64
KV = P + N             # 1101
KT = 9                 # kv tiles of 128
KVP = KT * 128         # 1152 padded
NT = N // 128          # 8 n tiles
CC = DIM // 128        # 8 contraction chunks
DR = CC // 2           # 4 double-row chunks (K=256 each)
OT = DIM // 128        # 8 output-channel tiles
LN_EPS = 1e-5
WSCALE = 256.0         # host premultiply on Wq/Wk so fp8 sees ~N(0,5) values
EXP_SCALE = 0.125 / (WSCALE * WSCALE)

LAST_EXEC_NS = None
_CACHE = {}


def _emit(tc):
    nc = tc.nc

    xcat8_d = nc.dram_tensor("xcat8", [DIM, KVP], E4, kind="ExternalInput").ap()
    xcatT_d = nc.dram_tensor("xcatT", [DIM, KVP], BF16, kind="ExternalInput").ap()
    wq8_d = nc.dram_tensor("wq8T", [DIM, DIM], E4, kind="ExternalInput").ap()
    wk8_d = nc.dram_tensor("wk8T", [DIM, DIM], E4, kind="ExternalInput").ap()
    wv_d = nc.dram_tensor("wvT", [DIM, DIM], BF16, kind="ExternalInput").ap()
    wp_d = nc.dram_tensor("wpT", [DIM, DIM], BF16, kind="ExternalInput").ap()
    tanhg_d = nc.dram_tensor("tanhg", [1, H], F32, kind="ExternalInput").ap()
    wbarbp_d = nc.dram_tensor("wbarbp", [2, DIM], BF16, kind="ExternalInput").ap()
    out_d = nc.dram_tensor("out", [N, DIM], BF16, kind="ExternalOutput").ap()

    xcat8_re = xcat8_d.rearrange("(j p) f -> p j f", p=128)
    xcat_re = xcatT_d.rearrange("(j p) f -> p j f", p=128)
    wq8_re = wq8_d.rearrange("(j p) o -> p j o", p=128)
    wk8_re = wk8_d.rearrange("(j p) o -> p j o", p=128)
    wv_re = wv_d.rearrange("(j p) o -> p j o", p=128)
    wp_re = wp_d.rearrange("(j p) o -> p j o", p=128)

    from contextlib import ExitStack

    with ExitStack() as top:
        consts = top.enter_context(tc.tile_pool(name="consts", bufs=1))
        acts = top.enter_context(tc.tile_pool(name="acts", bufs=1))
        ph1 = top.enter_context(tc.tile_pool(name="ph1", bufs=1))
        wstream = top.enter_context(tc.tile_pool(name="wstream", bufs=3))
        qkp = top.enter_context(tc.tile_pool(name="qkp", bufs=3))
        epool = top.enter_context(tc.tile_pool(name="epool", bufs=4))
        tpool = top.enter_context(tc.tile_pool(name="tmp", bufs=4))
        opool = top.enter_context(tc.tile_pool(name="outp", bufs=4))
        t1p = top.enter_context(tc.tile_pool(name="t1p", bufs=3))
        ps_proj = top.enter_context(tc.tile_pool(name="ps_proj", bufs=2, space="PSUM"))
        ps_scores = top.enter_context(
            tc.tile_pool(name="ps_scores", bufs=1, space="PSUM"))
        ps_av = top.enter_context(tc.tile_pool(name="ps_av", bufs=2, space="PSUM"))

        # ---- constants ----
        tanhg_sb = consts.tile([128, H], F32, tag="tanhg")
        eps_t = consts.tile([128, 1], F32, tag="eps")
        nc.vector.memset(eps_t, LN_EPS)
        ident = consts.tile([128, 128], BF16, tag="ident")
        make_identity(nc, ident)
        wbar_b = consts.tile([128, DIM], BF16, tag="wbar")
        bp_b = consts.tile([128, DIM], BF16, tag="bpb")

        # p-state warmup: the PE idles ~5us waiting for the first input
        # chunks and then runs pair 0's projections at the cold 1.2GHz
        # p-state (it only reaches 2.4GHz after ~3.4us of continuous work).
        # Burn that idle window on zero matmuls sized to drain just before
        # the real ones are data-ready.
        warm = consts.tile([128, 512], BF16, tag="warm")
        nc.vector.memset(warm, 0.0)
        wps = ps_av.tile([128, 512], F32, tag="avp")
        for i in range(8):
            nc.tensor.matmul(wps, warm[:, 0:128], warm,
                             start=(i == 0), stop=(i == 7))

        # ---- persistent activations ----
        vw_sb = acts.tile([128, KT, H, HD + 1], BF16, tag="vw")  # [kv-part, kv-tile, h, d+1]
        attn_sb = acts.tile([128, NT, H, HD], BF16, tag="attn")  # [n-part, n-tile, h, d]
        # transposed attn chunks for the output projection.  Chunk cc of the
        # channel dim is exactly head pair cc, so each pair's transposes run
        # right after its AV tails -- spread across the whole kernel instead
        # of bunched into the final phase.
        lt_sb = acts.tile([128, NT, CC, 128], BF16, tag="lt")    # [c-part, n-tile, pair, 2*d]
        # single persistent scores psum (4 banks): kvt alternates between the
        # two 1024-col halves and ONE exp drains each adjacent pair of them
        # as a 2048-col activation -- the ScalarE fixed cost (~293ns/inst)
        # amortizes over twice the columns.
        scps = ps_scores.tile([128, 2, 1024], F32, tag="pss")

        # input loads, c-chunk granular; fp8 xcat gates the q/k projections
        # (and hence scores + the exp stream) so it loads first; the bf16
        # xcat only feeds the v projection ~25us in.
        xcat8_sb = ph1.tile([128, CC, KVP], E4, tag="xcat8")
        xcatT_sb = ph1.tile([128, CC, KVP], BF16, tag="xcatT")
        # wv and wp share one slot: wv dies after the v projection, wp is
        # only needed from the output projection onwards
        wv_sb = ph1.tile([128, CC, DIM], BF16, tag="wvwp")
        # descriptor issue on an engine queue costs ~600ns each; spread the
        # startup-critical loads over the three DMA-capable queues
        w0q = wstream.tile([128, CC, 128], E4, tag="w")
        nc.scalar.dma_start(out=w0q, in_=wq8_re[:, :, 0:128])
        w0k = wstream.tile([128, CC, 128], E4, tag="w")
        nc.scalar.dma_start(out=w0k, in_=wk8_re[:, :, 0:128])
        xcat_q = [nc.gpsimd, nc.gpsimd, nc.gpsimd, nc.gpsimd,
                  nc.scalar, nc.sync, nc.sync, nc.sync]
        # two pieces per chunk, all first pieces ahead of all second pieces:
        # cols 0:640 cover everything the q half-0 (cols 77:589) and the
        # k split-0 (cols 0:512) projections read, so their matmuls -- and
        # the first scores/exps -- start sooner than with whole-chunk loads
        for cc in range(CC):
            xcat_q[cc].dma_start(out=xcat8_sb[:, cc, 0:640],
                                 in_=xcat8_re[:, cc, 0:640])
        for cc in range(CC):
            xcat_q[cc].dma_start(out=xcat8_sb[:, cc, 640:KVP],
                                 in_=xcat8_re[:, cc, 640:KVP])
        # bf16 xcat for the v projection: after the fp8 criticals
        for cc in range(CC):
            xcat_q[cc].dma_start(out=xcatT_sb[:, cc, :], in_=xcat_re[:, cc, :])

        # ---- q/k projections interleaved with their dependent head pairs,
        # so ScalarE (exp) fills while PE still runs projections ----
        last_rows = KV - (KT - 1) * 128  # 77

        def emit_vproj(kvts):
            # v projection into vw (head-interleaved), natural [kv, o] layout
            for kvt in kvts:
                for half in range(2):
                    ps = ps_proj.tile([128, 512], F32, tag="ps")
                    for cc in range(CC):
                        nc.tensor.matmul(
                            ps,
                            xcatT_sb[:, cc, kvt * 128:(kvt + 1) * 128],
                            wv_sb[:, cc, half * 512:(half + 1) * 512],
                            start=(cc == 0),
                            stop=(cc == CC - 1),
                        )
                    nc.vector.tensor_copy(
                        vw_sb[:, kvt, half * 8:(half + 1) * 8, 0:HD],
                        ps.rearrange("p (h d) -> p h d", d=HD),
                    )

        def emit_qk(ot, wtq=None, wtk=None):
            qt = qkp.tile([128, N], BF16, tag="qt")
            kt = qkp.tile([128, KVP], BF16, tag="kt")
            # pad keys (kv 1101:1152) are zero; scores psum partitions for
            # them are never read by the exp, but zero them for the checker
            nc.gpsimd.memset(kt[:, KV:KVP], 0.0)
            if wtq is None:
                wtq = wstream.tile([128, CC, 128], E4, tag="w")
                nc.sync.dma_start(out=wtq, in_=wq8_re[:, :, ot * 128:(ot + 1) * 128])
            for half in range(2):
                ps = ps_proj.tile([128, 512], F32, tag="ps")
                for j in range(DR):
                    nc.tensor.matmul(
                        ps,
                        wtq[:, 2 * j:2 * j + 2, :],
                        xcat8_sb[:, 2 * j:2 * j + 2,
                                 P + half * 512: P + (half + 1) * 512],
                        start=(j == 0),
                        stop=(j == DR - 1),
                        perf_mode=DRMODE,
                    )
                nc.vector.tensor_copy(qt[:, half * 512:(half + 1) * 512], ps)
            if wtk is None:
                wtk = wstream.tile([128, CC, 128], E4, tag="w")
                nc.sync.dma_start(out=wtk, in_=wk8_re[:, :, ot * 128:(ot + 1) * 128])
            for off in (0, 512):
                ps = ps_proj.tile([128, 512], F32, tag="ps")
                for j in range(DR):
                    nc.tensor.matmul(
                        ps,
                        wtk[:, 2 * j:2 * j + 2, :],
                        xcat8_sb[:, 2 * j:2 * j + 2, off:off + 512],
                        start=(j == 0),
                        stop=(j == DR - 1),
                        perf_mode=DRMODE,
                    )
                nc.vector.tensor_copy(kt[:, off:off + 512], ps)
            # 77-col text tail: DoubleRow loses below 128 free cols, so run
            # it as plain fp8 matmuls (bf16 rate)
            ps = ps_proj.tile([128, 512], F32, tag="ps")
            for cc in range(CC):
                nc.tensor.matmul(
                    ps[:, :last_rows],
                    wtk[:, cc, :],
                    xcat8_sb[:, cc, 1024:KV],
                    start=(cc == 0),
                    stop=(cc == CC - 1),
                )
            nc.vector.tensor_copy(kt[:, 1024:KV], ps[:, :last_rows])
            return qt, kt

        def emit_scores_pair(qt, kt):
            # Scores for the even/odd head pair.  Both heads of a (kv-tile,
            # n-half) share ONE [128,1024] psum tile: even head -> cols
            # 0:512 (bank A) on PE row-tile 0, odd head -> cols 512:1024
            # (bank B) on row-tile 64.  Disjoint row groups + disjoint psum
            # banks + a shared psum-reuse dependency means the two matmuls
            # issue back-to-back and execute concurrently on the PE.
            # E layout: [kv-part, kv-tile, n-half, head, 512], fp8.
            e2 = epool.tile([128, KT, 2, 2, 512], E4, tag="e")
            nc.gpsimd.memset(e2[:, KT - 1], 0.0)
            # half-major order so the AV of n-tiles 0-3 (which only needs
            # half 0) can start as soon as half 0's exps have drained
            for half in range(2):
                for kvt in range(KT):
                    sub = kvt % 2
                    nc.tensor.matmul(
                        scps[:, sub, 0:512],
                        kt[0:64, kvt * 128:(kvt + 1) * 128],
                        qt[0:64, half * 512:(half + 1) * 512],
                        start=True, stop=True,
                    )
                    nc.tensor.matmul(
                        scps[:, sub, 512:1024],
                        kt[64:128, kvt * 128:(kvt + 1) * 128],
                        qt[64:128, half * 512:(half + 1) * 512],
                        start=True, stop=True,
                    )
                    if kvt % 2 == 1:
                        nc.scalar.activation(
                            e2[:, kvt - 1:kvt + 1, half].rearrange(
                                "p a b c -> p a (b c)"),
                            scps,
                            AF.Exp, bias=0.0, scale=EXP_SCALE)
                # kvt 8 tail (77 live kv rows) drains alone
                nc.scalar.activation(
                    e2[:last_rows, KT - 1, half].rearrange("p b c -> p (b c)"),
                    scps[:last_rows, 0],
                    AF.Exp, bias=0.0, scale=EXP_SCALE)
            # first key column is gated separately; one memset per n-half so
            # the AV of n-tiles 0-3 only depends on half 0's exps
            nc.gpsimd.memset(e2[0:1, 0, 0], 0.0)
            nc.gpsimd.memset(e2[0:1, 0, 1], 0.0)
            return e2

        def eslice(e2, hh, kvt, nt):
            q, r = divmod(nt, 4)
            return e2[:, kvt, q, hh, r * 128:(r + 1) * 128]

        def emit_gv0(h):
            gv0 = tpool.tile([128, HD], BF16, tag="gv0")
            nc.gpsimd.partition_broadcast(gv0, vw_sb[0:1, 0, h, 0:HD])
            gv0s = tpool.tile([128, HD], F32, tag="gv0s")
            nc.vector.tensor_scalar_mul(gv0s, gv0, tanhg_sb[:, h:h + 1])
            return gv0s

        def emit_av_nt(h, e2, hh, nt, gv0s, alt, fixup_engine=None):
            if alt and nt % 2 == 1:
                avp = ps_proj.tile([128, HD + 1], F32, tag="ps")
            else:
                avp = ps_av.tile([128, HD + 1], F32, tag="avp")
            for kvt in range(KT):
                nc.tensor.matmul(
                    avp,
                    eslice(e2, hh, kvt, nt),
                    vw_sb[:, kvt, h, :],
                    start=(kvt == 0),
                    stop=(kvt == KT - 1),
                )
            rs = tpool.tile([128, 1], F32, tag="rs")
            nc.vector.reciprocal(rs, avp[:, HD:HD + 1])
            (fixup_engine or nc.vector).scalar_tensor_tensor(
                out=attn_sb[:, nt, h, :],
                in0=avp[:, 0:HD],
                scalar=rs,
                in1=gv0s,
                op0=OP.mult,
                op1=OP.add,
            )

        def emit_head_tail(h, e2, hh, alt=False):
            """Everything after E for head h: gate prep, AV + fixup.
            alt=True additionally cycles the (by-then idle) proj psum pool
            for deeper AV pipelining on the final head pairs."""
            gv0s = emit_gv0(h)
            for nt in range(NT):
                emit_av_nt(h, e2, hh, nt, gv0s, alt)

        def emit_pair_transposes(p):
            """Transpose attn heads (2p, 2p+1) for all n-tiles into LT.
            Runs right after pair p's tails, spreading the PE transposes
            and their psum evacuations across the whole kernel instead of
            the final phase.  (DVE: gpsimd cannot read PSUM on trn2, and
            ScalarE's queue is the exp stream here.)"""
            for nt in range(NT):
                pst = ps_av.tile([128, 128], BF16, tag="avp")
                nc.tensor.transpose(
                    pst,
                    attn_sb[:, nt, 2 * p:2 * p + 2, :].rearrange("p h d -> p (h d)"),
                    ident,
                )
                nc.vector.tensor_copy(lt_sb[:, nt, p, :], pst)

        # ---- LN stats per n-tile.  Only mu/rstd are computed here; the
        # normalization itself is folded into the projection's psum
        # evacuation, so the PE transposes/matmuls never wait on it. ----
        def emit_stats(nt):
            xa = attn_sb[:, nt].rearrange("p h d -> p (h d)")
            xs = xa.rearrange("p (s f) -> p s f", f=512)
            stats = tpool.tile([128, 2, 6], F32, tag="stats")
            for s in range(2):
                nc.vector.bn_stats(stats[:, s, :], xs[:, s, :])
            mv = tpool.tile([128, 2], F32, tag="mv")
            nc.vector.bn_aggr(mv, stats)
            # rstd = exp(-0.5*ln(var+eps)): stays on the natural_log_exp
            # activation table set (a sqrt would trigger a ~2.7us mid-kernel
            # table switch on ScalarE right as the projection phase starts)
            lv = tpool.tile([128, 1], F32, tag="lv")
            nc.scalar.activation(lv, mv[:, 1:2], AF.Ln, bias=eps_t, scale=1.0)
            rstd = tpool.tile([128, 1], F32, tag="rstd")
            nc.scalar.activation(rstd, lv, AF.Exp, bias=0.0, scale=-0.5)
            m2 = tpool.tile([128, 1], F32, tag="m2")
            nc.vector.tensor_scalar_mul(m2, mv[:, 0:1], rstd)
            # t1[n, o] = mu*rstd*colsum(Wpf)[o] - bp[o]
            t1 = t1p.tile([128, DIM], BF16, tag="t1")
            nc.vector.scalar_tensor_tensor(
                out=t1, in0=wbar_b, scalar=m2, in1=bp_b,
                op0=OP.mult, op1=OP.subtract,
            )
            return rstd, t1

        # pair 0's scores/exp are hoisted before the v projection so ScalarE
        # starts as early as possible
        # software pipeline: scores/exp run one head-pair ahead of the
        # AV/fixup tails so ScalarE never starves
        pend = []
        qt0, kt0 = emit_qk(0, w0q, w0k)
        # split sync/scalar: one queue alone delivers the last chunk too
        # late for the v-projection (keep gpsimd free for the e2 memsets;
        # the scalar queue's exp stream only starts at the first pair)
        for cc in range(CC):
            dmae = nc.sync if cc % 2 == 0 else nc.scalar
            dmae.dma_start(out=wv_sb[:, cc, :], in_=wv_re[:, cc, :])
        pend.append(emit_scores_pair(qt0, kt0))
        qt, kt = emit_qk(1)
        pend.append(emit_scores_pair(qt, kt))
        # ones column for the row-sum S (E rows for kv=0/pad are zeroed);
        # disjoint from the v-projection's columns, so set it up front
        nc.gpsimd.memset(vw_sb[:, :, :, HD:HD + 1], 1.0)
        # tanh(gate) + folded colsum(Wpf)/bp broadcasts: first consumers are
        # ~90us in, keep them off the startup-critical queues
        nc.sync.dma_start(out=tanhg_sb, in_=tanhg_d.to_broadcast([128, H]))
        nc.sync.dma_start(out=wbar_b, in_=wbarbp_d[0:1, :].to_broadcast([128, DIM]))
        nc.sync.dma_start(out=bp_b, in_=wbarbp_d[1:2, :].to_broadcast([128, DIM]))
        # pairs 2-3's scores BEFORE the v-projection: with the fp8 q/k
        # projections the PE reaches the v-projection much sooner, so four
        # banked pairs (~75us of exp work) keep ScalarE fed through it and
        # give the bf16-xcat/wv DMAs time to land
        qt, kt = emit_qk(2)
        pend.append(emit_scores_pair(qt, kt))
        qt, kt = emit_qk(3)
        pend.append(emit_scores_pair(qt, kt))
        emit_vproj(range(KT))
        wp_sb = ph1.tile([128, CC, DIM], BF16, tag="wvwp")
        for cc in range(CC):
            nc.sync.dma_start(out=wp_sb[:, cc, :], in_=wp_re[:, cc, :])
        ep = pend.pop(0)
        emit_head_tail(0, ep, 0)
        emit_head_tail(1, ep, 1)
        emit_pair_transposes(0)
        ep = pend.pop(0)
        emit_head_tail(2, ep, 0)
        emit_head_tail(3, ep, 1)
        emit_pair_transposes(1)
        done = 2
        for ot in range(4, OT):
            qt, kt = emit_qk(ot)
            pend.append(emit_scores_pair(qt, kt))
            ep = pend.pop(0)
            emit_head_tail(2 * done, ep, 0, alt=(ot >= OT - 2))
            emit_head_tail(2 * done + 1, ep, 1, alt=(ot >= OT - 2))
            emit_pair_transposes(done)
            done += 1

        def emit_outproj(nt, rstd, t1):
            # project the transposed raw-attn chunks (banked in LT across
            # the whole kernel): out[n, o] = (attn @ Wpf.T)*rstd - t1
            pp0 = ps_proj.tile([128, 512], F32, tag="ps")
            pp1 = ps_proj.tile([128, 512], F32, tag="ps")
            for cc in range(CC):
                nc.tensor.matmul(
                    pp0, lt_sb[:, nt, cc, :], wp_sb[:, cc, 0:512],
                    start=(cc == 0), stop=(cc == CC - 1),
                )
                nc.tensor.matmul(
                    pp1, lt_sb[:, nt, cc, :], wp_sb[:, cc, 512:1024],
                    start=(cc == 0), stop=(cc == CC - 1),
                )
            ot0 = opool.tile([128, 512], BF16, tag="ot")
            ot1 = opool.tile([128, 512], BF16, tag="ot")
            nc.vector.scalar_tensor_tensor(
                out=ot0, in0=pp0, scalar=rstd, in1=t1[:, 0:512],
                op0=OP.mult, op1=OP.subtract)
            nc.vector.scalar_tensor_tensor(
                out=ot1, in0=pp1, scalar=rstd, in1=t1[:, 512:1024],
                op0=OP.mult, op1=OP.subtract)
            # spread the 2MB of output across all three DMA rings
            out_q = [nc.sync, nc.gpsimd, nc.scalar]
            out_q[(2 * nt) % 3].dma_start(
                out=out_d[nt * 128:(nt + 1) * 128, 0:512], in_=ot0)
            out_q[(2 * nt + 1) % 3].dma_start(
                out=out_d[nt * 128:(nt + 1) * 128, 512:1024], in_=ot1)

        ep = pend.pop(0)
        emit_head_tail(2 * done, ep, 0, alt=True)
        emit_head_tail(2 * done + 1, ep, 1, alt=True)
        emit_pair_transposes(done)
        done += 1
        # final pair: nt-major AV with the LN stats and the output
        # projection of earlier n-tiles interleaved, so the PE stays on
        # projection matmuls while the DVE runs stats chains.
        ep = pend.pop(0)
        gv0s_e = emit_gv0(2 * done)
        gv0s_o = emit_gv0(2 * done + 1)
        # depth-2 pipeline: the stats chain of n-tile nt has the AV of nt+1
        # plus the projection of nt-1 of PE time to complete before the
        # evacuation of nt needs it
        L_q = []
        for nt in range(NT):
            emit_av_nt(2 * done, ep, 0, nt, gv0s_e, alt=False)
            emit_av_nt(2 * done + 1, ep, 1, nt, gv0s_o, alt=False)
            # last chunk's transpose for this n-tile (heads 14/15 just fixed
            # up); everything else is already banked in LT
            pst = ps_av.tile([128, 128], BF16, tag="avp")
            nc.tensor.transpose(
                pst,
                attn_sb[:, nt, 2 * done:2 * done + 2, :].rearrange(
                    "p h d -> p (h d)"),
                ident,
            )
            nc.vector.tensor_copy(lt_sb[:, nt, done, :], pst)
            rstd, t1 = emit_stats(nt)
            L_q.append((nt, rstd, t1))
            if len(L_q) > 2:
                emit_outproj(*L_q.pop(0))
        for item in L_q:
            emit_outproj(*item)


def build_program(with_bias=False):
    key = "nc"
    if key in _CACHE:
        return _CACHE[key]
    nc = bacc.Bacc("TRN2", target_bir_lowering=False, debug=False, num_devices=8,
                   enable_partition_id=False)
    with tile.TileContext(nc) as tc:
        _emit(tc)
    nc.compile()
    _CACHE[key] = nc
    return nc


def prep_inputs(x, x_text, Wq, Wk, Wv, gate, ln_g, ln_b, Wp, bp):
    """Host-side sharding/layout prep. Returns the 8 per-core input maps."""
    bf = ml_dtypes.bfloat16
    e4 = ml_dtypes.float8_e4m3
    x = np.asarray(x, np.float32)
    x_text = np.asarray(x_text, np.float32)
    xcat = np.concatenate([x_text, x], axis=1)          # [B, KV, DIM]
    xcatT = np.zeros((B, DIM, KVP), np.float32)
    xcatT[:, :, :KV] = xcat.transpose(0, 2, 1)
    xcat8 = np.clip(xcatT, -240, 240).astype(e4)
    xcatT = xcatT.astype(bf)
    wq8T = np.clip(np.asarray(Wq, np.float32).T * WSCALE, -240, 240).astype(e4)
    wk8T = np.clip(np.asarray(Wk, np.float32).T * WSCALE, -240, 240).astype(e4)
    wq8T = np.ascontiguousarray(wq8T)
    wk8T = np.ascontiguousarray(wk8T)
    wvT = np.ascontiguousarray(np.asarray(Wv, np.float32).T).astype(bf)
    # fold LayerNorm affine into the output projection:
    #   ((L - mu)*rstd*g + b) @ Wp.T + bp
    #     == (attn @ (Wp*g).T)*rstd - (mu*rstd*colsum(Wp*g) - (bp + Wp@b))
    Wp = np.asarray(Wp, np.float32)
    g = np.asarray(ln_g, np.float32).reshape(DIM)
    bvec = np.asarray(ln_b, np.float32).reshape(DIM)
    Wpf = Wp * g[None, :]
    bpf = np.asarray(bp, np.float32).reshape(DIM) + Wp @ bvec
    wpT = np.ascontiguousarray(Wpf.T).astype(bf)
    wbar = Wpf.sum(axis=1)                               # colsum over c, [DIM]
    wbarbp = np.stack([wbar, bpf]).astype(bf)            # [2, DIM]
    tanhg = np.tanh(np.asarray(gate, np.float32)).reshape(1, H).astype(np.float32)
    in_maps = []
    for b in range(B):
        in_maps.append({
            "xcat8": np.ascontiguousarray(xcat8[b]),
            "xcatT": np.ascontiguousarray(xcatT[b]),
            "wq8T": wq8T, "wk8T": wk8T, "wvT": wvT, "wpT": wpT,
            "tanhg": tanhg, "wbarbp": wbarbp,
        })
    return in_maps


def kernel(**inputs):
    global LAST_EXEC_NS
    in_maps = prep_inputs(**inputs)
    nc = build_program()
    trace = bool(int(os.environ.get("BASS_TRACE_RUN", "0")))
    res = run_bass_kernel_spmd(
        nc, in_maps, core_ids=list(range(8)), trace=trace,
    )
    LAST_EXEC_NS = res.exec_time_ns
    out = np.stack([r["out"] for r in res.results], axis=0)
    return out.astype(np.float32)


# revision 16
# speedup vs baseline: 1.1869x; 1.1869x over previous
"""Trainium2 Bass kernel for nn_Attention_45724221833663 (sparse_attention).

Strategy: data-parallel over batch B=8 across the 8 NeuronCores (one batch
element per core).  Matmul dtype mix: the q/k projections run in fp8e4
DoubleRow (2 contraction rows per PE cell -> half the matmul passes), the
softmax weights E are stored fp8 (AV = fp8 stationary x bf16 moving), and
everything whose error would reach the output directly (v projection,
scores operands, AV values, output projection) stays bf16 with fp32 PSUM.
Validated end-to-end: amax rel err ~8e-3 vs fp32 reference (tol 2e-2).

Per-core dataflow (host pre-transposes weights/x while sharding; ln_g is
folded into Wp, ln_b into bp):
  xcat8  [c=1024, kvp=1152] fp8  (concat(x_text,x).T, q/k weights x256 on
         host, the 1/65536 is folded into the exp scale)
  xcatT  [c, kvp] bf16           (v projection only)
  vw     [kvp, h, 65] bf16 = (xcatT.T @ WvT) per head + ones column
  qT/kT via DoubleRow fp8: 4 passes of K=256 instead of 8 of K=128
  per head pair (even head on PE row-tile 0, odd on row-tile 64):
    scores for both heads go into ONE [128,1024] psum tile per (kv-tile,
    n-half): disjoint PE row groups + disjoint psum banks -> the two
    matmuls execute concurrently.
    E = exp(scoresT/(8*65536)) -> fp8 e2 tile; kv=0 row and pad rows zeroed
    avp[n,0:65] = sum_kv E[kv,..] @ vw[kv,h,:]  (col 64 = S[n])
    attn[n, h*64:+64] = avp[:, :64]*(1/S) + tanh(g_h)*v_h[kv=0]
  Final head pair runs n-tile-major with LN stats + output projection
  interleaved.  The projection consumes RAW attn (transposed via PE), and
  LayerNorm is applied algebraically at psum evacuation:
      out = (attnT @ Wpf) * rstd - (mu*rstd*colsum(Wpf) - bp)
  so the PE transpose/matmul chain never waits on the LN stats.  rstd is
  computed as exp(-0.5*ln(var+eps)) so ScalarE stays on the
  natural_log_exp table set (no mid-kernel table switch).  Transposes are
  hoisted 2 chunks ahead of their matmuls and their psum->sbuf copies run
  on ScalarE (idle there: all exps are done by that phase).  Output is
  written bf16 and upcast on the host.
"""

import os
import numpy as np
import ml_dtypes

import concourse.bacc as bacc
import concourse.tile as tile
from concourse import mybir
from concourse.masks import make_identity
from concourse.bass_utils import run_bass_kernel_spmd

F32 = mybir.dt.float32
BF16 = mybir.dt.bfloat16
E4 = mybir.dt.float8e4
AF = mybir.ActivationFunctionType
OP = mybir.AluOpType
DRMODE = mybir.MatmulPerfMode.DoubleRow

B, N, P, DIM, H = 8, 1024, 77, 1024, 16
HD = DIM // H          # 64
KV = P + N             # 1101
KT = 9                 # kv tiles of 128
KVP = KT * 128         # 1152 padded
NT = N // 128          # 8 n tiles
CC = DIM // 128        # 8 contraction chunks
DR = CC // 2           # 4 double-row chunks (K=256 each)
OT = DIM // 128        # 8 output-channel tiles
LN_EPS = 1e-5
WSCALE = 256.0         # host premultiply on Wq/Wk so fp8 sees ~N(0,5) values
EXP_SCALE = 0.125 / (WSCALE * WSCALE)

LAST_EXEC_NS = None
_CACHE = {}


def _emit(tc):
    nc = tc.nc

    xcat8_d = nc.dram_tensor("xcat8", [DIM, KVP], E4, kind="ExternalInput").ap()
    xcatT_d = nc.dram_tensor("xcatT", [DIM, KVP], BF16, kind="ExternalInput").ap()
    wq8_d = nc.dram_tensor("wq8T", [DIM, DIM], E4, kind="ExternalInput").ap()
    wk8_d = nc.dram_tensor("wk8T", [DIM, DIM], E4, kind="ExternalInput").ap()
    wv_d = nc.dram_tensor("wvT", [DIM, DIM], BF16, kind="ExternalInput").ap()
    wp_d = nc.dram_tensor("wpT", [DIM, DIM], BF16, kind="ExternalInput").ap()
    tanhg_d = nc.dram_tensor("tanhg", [1, H], F32, kind="ExternalInput").ap()
    wbarbp_d = nc.dram_tensor("wbarbp", [2, DIM], BF16, kind="ExternalInput").ap()
    out_d = nc.dram_tensor("out", [N, DIM], BF16, kind="ExternalOutput").ap()

    xcat8_re = xcat8_d.rearrange("(j p) f -> p j f", p=128)
    xcat_re = xcatT_d.rearrange("(j p) f -> p j f", p=128)
    wq8_re = wq8_d.rearrange("(j p) o -> p j o", p=128)
    wk8_re = wk8_d.rearrange("(j p) o -> p j o", p=128)
    wv_re = wv_d.rearrange("(j p) o -> p j o", p=128)
    wp_re = wp_d.rearrange("(j p) o -> p j o", p=128)

    from contextlib import ExitStack

    with ExitStack() as top:
        consts = top.enter_context(tc.tile_pool(name="consts", bufs=1))
        acts = top.enter_context(tc.tile_pool(name="acts", bufs=1))
        ph1 = top.enter_context(tc.tile_pool(name="ph1", bufs=1))
        wstream = top.enter_context(tc.tile_pool(name="wstream", bufs=3))
        qkp = top.enter_context(tc.tile_pool(name="qkp", bufs=3))
        epool = top.enter_context(tc.tile_pool(name="epool", bufs=4))
        tpool = top.enter_context(tc.tile_pool(name="tmp", bufs=4))
        opool = top.enter_context(tc.tile_pool(name="outp", bufs=4))
        t1p = top.enter_context(tc.tile_pool(name="t1p", bufs=3))
        ps_proj = top.enter_context(tc.tile_pool(name="ps_proj", bufs=2, space="PSUM"))
        ps_scores = top.enter_context(
            tc.tile_pool(name="ps_scores", bufs=2, space="PSUM"))
        ps_av = top.enter_context(tc.tile_pool(name="ps_av", bufs=2, space="PSUM"))

        # ---- constants ----
        tanhg_sb = consts.tile([128, H], F32, tag="tanhg")
        eps_t = consts.tile([128, 1], F32, tag="eps")
        nc.vector.memset(eps_t, LN_EPS)
        ident = consts.tile([128, 128], BF16, tag="ident")
        make_identity(nc, ident)
        wbar_b = consts.tile([128, DIM], BF16, tag="wbar")
        bp_b = consts.tile([128, DIM], BF16, tag="bpb")

        # p-state warmup: the PE idles ~5us waiting for the first input
        # chunks and then runs pair 0's projections at the cold 1.2GHz
        # p-state (it only reaches 2.4GHz after ~3.4us of continuous work).
        # Burn that idle window on zero matmuls sized to drain just before
        # the real ones are data-ready.
        warm = consts.tile([128, 512], BF16, tag="warm")
        nc.vector.memset(warm, 0.0)
        wps = ps_av.tile([128, 512], F32, tag="avp")
        for i in range(8):
            nc.tensor.matmul(wps, warm[:, 0:128], warm,
                             start=(i == 0), stop=(i == 7))

        # ---- persistent activations ----
        vw_sb = acts.tile([128, KT, H, HD + 1], BF16, tag="vw")  # [kv-part, kv-tile, h, d+1]
        attn_sb = acts.tile([128, NT, H, HD], BF16, tag="attn")  # [n-part, n-tile, h, d]
        # transposed attn chunks for the output projection.  Chunk cc of the
        # channel dim is exactly head pair cc, so each pair's transposes run
        # right after its AV tails -- spread across the whole kernel instead
        # of bunched into the final phase.
        lt_sb = acts.tile([128, NT, CC, 128], BF16, tag="lt")    # [c-part, n-tile, pair, 2*d]

        # input loads, c-chunk granular; fp8 xcat gates the q/k projections
        # (and hence scores + the exp stream) so it loads first; the bf16
        # xcat only feeds the v projection ~25us in.
        xcat8_sb = ph1.tile([128, CC, KVP], E4, tag="xcat8")
        xcatT_sb = ph1.tile([128, CC, KVP], BF16, tag="xcatT")
        # wv and wp share one slot: wv dies after the v projection, wp is
        # only needed from the output projection onwards
        wv_sb = ph1.tile([128, CC, DIM], BF16, tag="wvwp")
        # descriptor issue on an engine queue costs ~600ns each; spread the
        # startup-critical loads over the three DMA-capable queues
        w0q = wstream.tile([128, CC, 128], E4, tag="w")
        nc.scalar.dma_start(out=w0q, in_=wq8_re[:, :, 0:128])
        w0k = wstream.tile([128, CC, 128], E4, tag="w")
        nc.scalar.dma_start(out=w0k, in_=wk8_re[:, :, 0:128])
        xcat_q = [nc.gpsimd, nc.gpsimd, nc.gpsimd, nc.gpsimd,
                  nc.scalar, nc.sync, nc.sync, nc.sync]
        # two pieces per chunk, all first pieces ahead of all second pieces:
        # cols 0:640 cover everything the q half-0 (cols 77:589) and the
        # k split-0 (cols 0:512) projections read, so their matmuls -- and
        # the first scores/exps -- start sooner than with whole-chunk loads
        for cc in range(CC):
            xcat_q[cc].dma_start(out=xcat8_sb[:, cc, 0:640],
                                 in_=xcat8_re[:, cc, 0:640])
        for cc in range(CC):
            xcat_q[cc].dma_start(out=xcat8_sb[:, cc, 640:KVP],
                                 in_=xcat8_re[:, cc, 640:KVP])
        # bf16 xcat for the v projection: after the fp8 criticals
        for cc in range(CC):
            xcat_q[cc].dma_start(out=xcatT_sb[:, cc, :], in_=xcat_re[:, cc, :])

        # ---- q/k projections interleaved with their dependent head pairs,
        # so ScalarE (exp) fills while PE still runs projections ----
        last_rows = KV - (KT - 1) * 128  # 77

        def emit_vproj(kvts):
            # v projection into vw (head-interleaved), natural [kv, o] layout
            for kvt in kvts:
                for half in range(2):
                    ps = ps_proj.tile([128, 512], F32, tag="ps")
                    for cc in range(CC):
                        nc.tensor.matmul(
                            ps,
                            xcatT_sb[:, cc, kvt * 128:(kvt + 1) * 128],
                            wv_sb[:, cc, half * 512:(half + 1) * 512],
                            start=(cc == 0),
                            stop=(cc == CC - 1),
                        )
                    nc.vector.tensor_copy(
                        vw_sb[:, kvt, half * 8:(half + 1) * 8, 0:HD],
                        ps.rearrange("p (h d) -> p h d", d=HD),
                    )

        def emit_qk(ot, wtq=None, wtk=None):
            qt = qkp.tile([128, N], BF16, tag="qt")
            kt = qkp.tile([128, KVP], BF16, tag="kt")
            # pad keys (kv 1101:1152) are zero; scores psum partitions for
            # them are never read by the exp, but zero them for the checker
            nc.gpsimd.memset(kt[:, KV:KVP], 0.0)
            if wtq is None:
                wtq = wstream.tile([128, CC, 128], E4, tag="w")
                nc.sync.dma_start(out=wtq, in_=wq8_re[:, :, ot * 128:(ot + 1) * 128])
            for half in range(2):
                ps = ps_proj.tile([128, 512], F32, tag="ps")
                for j in range(DR):
                    nc.tensor.matmul(
                        ps,
                        wtq[:, 2 * j:2 * j + 2, :],
                        xcat8_sb[:, 2 * j:2 * j + 2,
                                 P + half * 512: P + (half + 1) * 512],
                        start=(j == 0),
                        stop=(j == DR - 1),
                        perf_mode=DRMODE,
                    )
                nc.vector.tensor_copy(qt[:, half * 512:(half + 1) * 512], ps)
            if wtk is None:
                wtk = wstream.tile([128, CC, 128], E4, tag="w")
                nc.sync.dma_start(out=wtk, in_=wk8_re[:, :, ot * 128:(ot + 1) * 128])
            for off in (0, 512):
                ps = ps_proj.tile([128, 512], F32, tag="ps")
                for j in range(DR):
                    nc.tensor.matmul(
                        ps,
                        wtk[:, 2 * j:2 * j + 2, :],
                        xcat8_sb[:, 2 * j:2 * j + 2, off:off + 512],
                        start=(j == 0),
                        stop=(j == DR - 1),
                        perf_mode=DRMODE,
                    )
                nc.vector.tensor_copy(kt[:, off:off + 512], ps)
            # 77-col text tail: DoubleRow loses below 128 free cols, so run
            # it as plain fp8 matmuls (bf16 rate)
            ps = ps_proj.tile([128, 512], F32, tag="ps")
            for cc in range(CC):
                nc.tensor.matmul(
                    ps[:, :last_rows],
                    wtk[:, cc, :],
                    xcat8_sb[:, cc, 1024:KV],
                    start=(cc == 0),
                    stop=(cc == CC - 1),
                )
            nc.vector.tensor_copy(kt[:, 1024:KV], ps[:, :last_rows])
            return qt, kt

        def emit_scores_pair(qt, kt):
            # Scores for the even/odd head pair.  Both heads of a (kv-tile,
            # n-half) share ONE [128,1024] psum tile: even head -> cols
            # 0:512 (bank A) on PE row-tile 0, odd head -> cols 512:1024
            # (bank B) on row-tile 64.  Disjoint row groups + disjoint psum
            # banks + a shared psum-reuse dependency means the two matmuls
            # issue back-to-back and execute concurrently on the PE.
            # E layout: [kv-part, kv-tile, n-half, head, 512], fp8.
            e2 = epool.tile([128, KT, 2, 2, 512], E4, tag="e")
            nc.gpsimd.memset(e2[:, KT - 1], 0.0)
            # half-major order so the AV of n-tiles 0-3 (which only needs
            # half 0) can start as soon as half 0's exps have drained
            for half in range(2):
                for kvt in range(KT):
                    rows = last_rows if kvt == KT - 1 else 128
                    ps = ps_scores.tile([128, 1024], F32, tag="pss")
                    nc.tensor.matmul(
                        ps[:, 0:512],
                        kt[0:64, kvt * 128:(kvt + 1) * 128],
                        qt[0:64, half * 512:(half + 1) * 512],
                        start=True, stop=True,
                    )
                    nc.tensor.matmul(
                        ps[:, 512:1024],
                        kt[64:128, kvt * 128:(kvt + 1) * 128],
                        qt[64:128, half * 512:(half + 1) * 512],
                        start=True, stop=True,
                    )
                    nc.scalar.activation(
                        e2[:rows, kvt, half], ps[:rows], AF.Exp,
                        bias=0.0, scale=EXP_SCALE)
            # first key column is gated separately; one memset per n-half so
            # the AV of n-tiles 0-3 only depends on half 0's exps
            nc.gpsimd.memset(e2[0:1, 0, 0], 0.0)
            nc.gpsimd.memset(e2[0:1, 0, 1], 0.0)
            return e2

        def eslice(e2, hh, kvt, nt):
            q, r = divmod(nt, 4)
            return e2[:, kvt, q, hh, r * 128:(r + 1) * 128]

        def emit_gv0(h):
            gv0 = tpool.tile([128, HD], BF16, tag="gv0")
            nc.gpsimd.partition_broadcast(gv0, vw_sb[0:1, 0, h, 0:HD])
            gv0s = tpool.tile([128, HD], F32, tag="gv0s")
            nc.vector.tensor_scalar_mul(gv0s, gv0, tanhg_sb[:, h:h + 1])
            return gv0s

        def emit_av_nt(h, e2, hh, nt, gv0s, alt, fixup_engine=None):
            if alt and nt % 2 == 1:
                avp = ps_proj.tile([128, HD + 1], F32, tag="ps")
            else:
                avp = ps_av.tile([128, HD + 1], F32, tag="avp")
            for kvt in range(KT):
                nc.tensor.matmul(
                    avp,
                    eslice(e2, hh, kvt, nt),
                    vw_sb[:, kvt, h, :],
                    start=(kvt == 0),
                    stop=(kvt == KT - 1),
                )
            rs = tpool.tile([128, 1], F32, tag="rs")
            nc.vector.reciprocal(rs, avp[:, HD:HD + 1])
            (fixup_engine or nc.vector).scalar_tensor_tensor(
                out=attn_sb[:, nt, h, :],
                in0=avp[:, 0:HD],
                scalar=rs,
                in1=gv0s,
                op0=OP.mult,
                op1=OP.add,
            )

        def emit_head_tail(h, e2, hh, alt=False):
            """Everything after E for head h: gate prep, AV + fixup.
            alt=True additionally cycles the (by-then idle) proj psum pool
            for deeper AV pipelining on the final head pairs."""
            gv0s = emit_gv0(h)
            for nt in range(NT):
                emit_av_nt(h, e2, hh, nt, gv0s, alt)

        def emit_pair_transposes(p):
            """Transpose attn heads (2p, 2p+1) for all n-tiles into LT.
            Runs right after pair p's tails, spreading the PE transposes
            and their psum evacuations across the whole kernel instead of
            the final phase.  (DVE: gpsimd cannot read PSUM on trn2, and
            ScalarE's queue is the exp stream here.)"""
            for nt in range(NT):
                pst = ps_av.tile([128, 128], BF16, tag="avp")
                nc.tensor.transpose(
                    pst,
                    attn_sb[:, nt, 2 * p:2 * p + 2, :].rearrange("p h d -> p (h d)"),
                    ident,
                )
                nc.vector.tensor_copy(lt_sb[:, nt, p, :], pst)

        # ---- LN stats per n-tile.  Only mu/rstd are computed here; the
        # normalization itself is folded into the projection's psum
        # evacuation, so the PE transposes/matmuls never wait on it. ----
        def emit_stats(nt):
            xa = attn_sb[:, nt].rearrange("p h d -> p (h d)")
            xs = xa.rearrange("p (s f) -> p s f", f=512)
            stats = tpool.tile([128, 2, 6], F32, tag="stats")
            for s in range(2):
                nc.vector.bn_stats(stats[:, s, :], xs[:, s, :])
            mv = tpool.tile([128, 2], F32, tag="mv")
            nc.vector.bn_aggr(mv, stats)
            # rstd = exp(-0.5*ln(var+eps)): stays on the natural_log_exp
            # activation table set (a sqrt would trigger a ~2.7us mid-kernel
            # table switch on ScalarE right as the projection phase starts)
            lv = tpool.tile([128, 1], F32, tag="lv")
            nc.scalar.activation(lv, mv[:, 1:2], AF.Ln, bias=eps_t, scale=1.0)
            rstd = tpool.tile([128, 1], F32, tag="rstd")
            nc.scalar.activation(rstd, lv, AF.Exp, bias=0.0, scale=-0.5)
            m2 = tpool.tile([128, 1], F32, tag="m2")
            nc.vector.tensor_scalar_mul(m2, mv[:, 0:1], rstd)
            # t1[n, o] = mu*rstd*colsum(Wpf)[o] - bp[o]
            t1 = t1p.tile([128, DIM], BF16, tag="t1")
            nc.vector.scalar_tensor_tensor(
                out=t1, in0=wbar_b, scalar=m2, in1=bp_b,
                op0=OP.mult, op1=OP.subtract,
            )
            return rstd, t1

        # pair 0's scores/exp are hoisted before the v projection so ScalarE
        # starts as early as possible
        # software pipeline: scores/exp run one head-pair ahead of the
        # AV/fixup tails so ScalarE never starves
        pend = []
        qt0, kt0 = emit_qk(0, w0q, w0k)
        # split sync/scalar: one queue alone delivers the last chunk too
        # late for the v-projection (keep gpsimd free for the e2 memsets;
        # the scalar queue's exp stream only starts at the first pair)
        for cc in range(CC):
            dmae = nc.sync if cc % 2 == 0 else nc.scalar
            dmae.dma_start(out=wv_sb[:, cc, :], in_=wv_re[:, cc, :])
        pend.append(emit_scores_pair(qt0, kt0))
        qt, kt = emit_qk(1)
        pend.append(emit_scores_pair(qt, kt))
        # ones column for the row-sum S (E rows for kv=0/pad are zeroed);
        # disjoint from the v-projection's columns, so set it up front
        nc.gpsimd.memset(vw_sb[:, :, :, HD:HD + 1], 1.0)
        # tanh(gate) + folded colsum(Wpf)/bp broadcasts: first consumers are
        # ~90us in, keep them off the startup-critical queues
        nc.sync.dma_start(out=tanhg_sb, in_=tanhg_d.to_broadcast([128, H]))
        nc.sync.dma_start(out=wbar_b, in_=wbarbp_d[0:1, :].to_broadcast([128, DIM]))
        nc.sync.dma_start(out=bp_b, in_=wbarbp_d[1:2, :].to_broadcast([128, DIM]))
        # pairs 2-3's scores BEFORE the v-projection: with the fp8 q/k
        # projections the PE reaches the v-projection much sooner, so four
        # banked pairs (~75us of exp work) keep ScalarE fed through it and
        # give the bf16-xcat/wv DMAs time to land
        qt, kt = emit_qk(2)
        pend.append(emit_scores_pair(qt, kt))
        qt, kt = emit_qk(3)
        pend.append(emit_scores_pair(qt, kt))
        emit_vproj(range(KT))
        wp_sb = ph1.tile([128, CC, DIM], BF16, tag="wvwp")
        for cc in range(CC):
            nc.sync.dma_start(out=wp_sb[:, cc, :], in_=wp_re[:, cc, :])
        ep = pend.pop(0)
        emit_head_tail(0, ep, 0)
        emit_head_tail(1, ep, 1)
        emit_pair_transposes(0)
        ep = pend.pop(0)
        emit_head_tail(2, ep, 0)
        emit_head_tail(3, ep, 1)
        emit_pair_transposes(1)
        done = 2
        for ot in range(4, OT):
            qt, kt = emit_qk(ot)
            pend.append(emit_scores_pair(qt, kt))
            ep = pend.pop(0)
            emit_head_tail(2 * done, ep, 0, alt=(ot >= OT - 2))
            emit_head_tail(2 * done + 1, ep, 1, alt=(ot >= OT - 2))
            emit_pair_transposes(done)
            done += 1

        def emit_outproj(nt, rstd, t1):
            # project the transposed raw-attn chunks (banked in LT across
            # the whole kernel): out[n, o] = (attn @ Wpf.T)*rstd - t1
            pp0 = ps_proj.tile([128, 512], F32, tag="ps")
            pp1 = ps_proj.tile([128, 512], F32, tag="ps")
            for cc in range(CC):
                nc.tensor.matmul(
                    pp0, lt_sb[:, nt, cc, :], wp_sb[:, cc, 0:512],
                    start=(cc == 0), stop=(cc == CC - 1),
                )
                nc.tensor.matmul(
                    pp1, lt_sb[:, nt, cc, :], wp_sb[:, cc, 512:1024],
                    start=(cc == 0), stop=(cc == CC - 1),
                )
            ot0 = opool.tile([128, 512], BF16, tag="ot")
            ot1 = opool.tile([128, 512], BF16, tag="ot")
            nc.vector.scalar_tensor_tensor(
                out=ot0, in0=pp0, scalar=rstd, in1=t1[:, 0:512],
                op0=OP.mult, op1=OP.subtract)
            nc.vector.scalar_tensor_tensor(
                out=ot1, in0=pp1, scalar=rstd, in1=t1[:, 512:1024],
                op0=OP.mult, op1=OP.subtract)
            # spread the 2MB of output across all three DMA rings
            out_q = [nc.sync, nc.gpsimd, nc.scalar]
            out_q[(2 * nt) % 3].dma_start(
                out=out_d[nt * 128:(nt + 1) * 128, 0:512], in_=ot0)
            out_q[(2 * nt + 1) % 3].dma_start(
                out=out_d[nt * 128:(nt + 1) * 128, 512:1024], in_=ot1)

        ep = pend.pop(0)
        emit_head_tail(2 * done, ep, 0, alt=True)
        emit_head_tail(2 * done + 1, ep, 1, alt=True)
        emit_pair_transposes(done)
        done += 1
        # final pair: nt-major AV with the LN stats and the output
        # projection of earlier n-tiles interleaved, so the PE stays on
        # projection matmuls while the DVE runs stats chains.
        ep = pend.pop(0)
        gv0s_e = emit_gv0(2 * done)
        gv0s_o = emit_gv0(2 * done + 1)
        # depth-2 pipeline: the stats chain of n-tile nt has the AV of nt+1
        # plus the projection of nt-1 of PE time to complete before the
        # evacuation of nt needs it
        L_q = []
        for nt in range(NT):
            emit_av_nt(2 * done, ep, 0, nt, gv0s_e, alt=False)
            emit_av_nt(2 * done + 1, ep, 1, nt, gv0s_o, alt=False)
            # last chunk's transpose for this n-tile (heads 14/15 just fixed
            # up); everything else is already banked in LT
            pst = ps_av.tile([128, 128], BF16, tag="avp")
            nc.tensor.transpose(
                pst,
                attn_sb[:, nt, 2 * done:2 * done + 2, :].rearrange(
                    "p h d -> p (h d)"),
                ident,
            )
            nc.vector.tensor_copy(lt_sb[:, nt, done, :], pst)
            rstd, t1 = emit_stats(nt)
            L_q.append((nt, rstd, t1))
            if len(L_q) > 2:
                emit_outproj(*L_q.pop(0))
        for item in L_q:
            emit_outproj(*item)


def build_program(with_bias=False):
    key = "nc"
    if key in _CACHE:
        return _CACHE[key]
    nc = bacc.Bacc("TRN2", target_bir_lowering=False, debug=False, num_devices=8,
                   enable_partition_id=False)
    with tile.TileContext(nc) as tc:
        _emit(tc)
    nc.compile()
    _CACHE[key] = nc
    return nc


def prep_inputs(x, x_text, Wq, Wk, Wv, gate, ln_g, ln_b, Wp, bp):
    """Host-side sharding/layout prep. Returns the 8 per-core input maps."""
    bf = ml_dtypes.bfloat16
    e4 = ml_dtypes.float8_e4m3
    x = np.asarray(x, np.float32)
    x_text = np.asarray(x_text, np.float32)
    xcat = np.concatenate([x_text, x], axis=1)          # [B, KV, DIM]
    xcatT = np.zeros((B, DIM, KVP), np.float32)
    xcatT[:, :, :KV] = xcat.transpose(0, 2, 1)
    xcat8 = np.clip(xcatT, -240, 240).astype(e4)
    xcatT = xcatT.astype(bf)
    wq8T = np.clip(np.asarray(Wq, np.float32).T * WSCALE, -240, 240).astype(e4)
    wk8T = np.clip(np.asarray(Wk, np.float32).T * WSCALE, -240, 240).astype(e4)
    wq8T = np.ascontiguousarray(wq8T)
    wk8T = np.ascontiguousarray(wk8T)
    wvT = np.ascontiguousarray(np.asarray(Wv, np.float32).T).astype(bf)
    # fold LayerNorm affine into the output projection:
    #   ((L - mu)*rstd*g + b) @ Wp.T + bp
    #     == (attn @ (Wp*g).T)*rstd - (mu*rstd*colsum(Wp*g) - (bp + Wp@b))
    Wp = np.asarray(Wp, np.float32)
    g = np.asarray(ln_g, np.float32).reshape(DIM)
    bvec = np.asarray(ln_b, np.float32).reshape(DIM)
    Wpf = Wp * g[None, :]
    bpf = np.asarray(bp, np.float32).reshape(DIM) + Wp @ bvec
    wpT = np.ascontiguousarray(Wpf.T).astype(bf)
    wbar = Wpf.sum(axis=1)                               # colsum over c, [DIM]
    wbarbp = np.stack([wbar, bpf]).astype(bf)            # [2, DIM]
    tanhg = np.tanh(np.asarray(gate, np.float32)).reshape(1, H).astype(np.float32)
    in_maps = []
    for b in range(B):
        in_maps.append({
            "xcat8": np.ascontiguousarray(xcat8[b]),
            "xcatT": np.ascontiguousarray(xcatT[b]),
            "wq8T": wq8T, "wk8T": wk8T, "wvT": wvT, "wpT": wpT,
            "tanhg": tanhg, "wbarbp": wbarbp,
        })
    return in_maps


def kernel(**inputs):
    global LAST_EXEC_NS
    in_maps = prep_inputs(**inputs)
    nc = build_program()
    trace = bool(int(os.environ.get("BASS_TRACE_RUN", "0")))
    res = run_bass_kernel_spmd(
        nc, in_maps, core_ids=list(range(8)), trace=trace,
    )
    LAST_EXEC_NS = res.exec_time_ns
    out = np.stack([r["out"] for r in res.results], axis=0)
    return out.astype(np.float32)
